# revision 16
# baseline (speedup 1.0000x reference)
"""Trainium2 Bass kernel for the YOLO-style DetectHead decode (nms_detection).

Contract: kernel(out1, out2, out3) -> (boxes, confs, depth), matching the
jax reference decode exactly (full 64-batch inputs in, full outputs out).

Strategy
--------
Pure data parallel over 8 NeuronCores: 8 batches per core. On-core, the
decode is laid out *detection-major*: every (batch, anchor) group's H*W
cells are split as HW = Q * 117 across Q SBUF partitions with 117
detections per partition (HW is 64/16/4 * 117 for the three scales, so
the split is exact with zero padding). Input planes land per partition as
contiguous 468-byte runs, all math runs as a handful of full-width
ACT/DVE ops per tile, and the [N,4]/[N,8] interleaved outputs are formed
directly by strided DVE writes - no transposes anywhere.

Sigmoid is computed as 0.5*tanh(x/2)+0.5 (tanh and exp share one ACT
table set; sigmoid+exp would force a ~2.7us table switch per op), with
the affine folded into fused scalar_tensor_tensor ops:
  u     = tanh(tx/2)*(0.5/W) + (gx+0.5)/W          (grid constant folded)
  ew    = exp(tw + ln(anchor_w/stride/(2W)))        (bias folded into exp)
  x1,x2 = u -/+ ew   (strided write into [.., j*4+comp])
  conf  = (tanh(cls/2) + 1) * (0.25*tanh(det/2)+0.25)   (one fused op)
  depth = raw copy (pure DMA from the loaded tile)
"""

from contextlib import ExitStack

import numpy as np

import concourse.bass as bass
import concourse.tile as tile
from concourse import bacc, mybir
from concourse.alu_op_type import AluOpType
from concourse.bass_utils import run_bass_kernel_spmd

N_CORES = 8
BC = 8          # batches per core (64 / 8)
CLSN = 8        # num classes
J = 117         # detections per partition (common factor of all three H*W)
F32 = mybir.dt.float32

# per scale: H, W, stride, anchor pairs (raw), Q = HW // J
SCALES = [
    dict(H=48, W=156, stride=8,  A=[(12, 16), (19, 36), (40, 28)],    Q=64),
    dict(H=24, W=78,  stride=16, A=[(36, 75), (76, 55), (72, 146)],   Q=16),
    dict(H=12, W=39,  stride=32, A=[(142, 110), (192, 243), (459, 401)], Q=4),
]
HWS = [s["H"] * s["W"] for s in SCALES]          # 7488, 1872, 468
NDET = 3 * sum(HWS)                              # 29484
NOFF = [0, 3 * HWS[0], 3 * HWS[0] + 3 * HWS[1]]  # per-scale offset in N
CW = 240                                         # const tile width


def _store_runs(s, groups):
    """Split a tile's groups into same-anchor, consecutive-b runs.
    Returns [(a, b_start, n_groups, group_index_start), ...]."""
    out = []
    i = 0
    while i < len(groups):
        a, b0 = groups[i]
        j = i
        while j + 1 < len(groups) and groups[j + 1] == (a, b0 + (j + 1 - i)):
            j += 1
        out.append((a, b0, j - i + 1, i))
        i = j + 1
    return out


def _tile_plan():
    """Yield (scale_idx, groups, Q, P) per SBUF tile; groups = [(a, b), ...]."""
    plan = []
    for a in range(3):                      # scale 0: 12 tiles of 2 groups
        for i in range(4):
            plan.append((0, [(a, 2 * i), (a, 2 * i + 1)], 64))
    for a in range(3):                      # scale 1: 3 tiles of 8 groups
        plan.append((1, [(a, b) for b in range(BC)], 16))
    plan.append((2, [(a, b) for a in range(3) for b in range(BC)], 4))
    return [(s, gs, q, len(gs) * q) for (s, gs, q) in plan]


def _build_consts():
    """Three [128, CW] f32 const blocks: grid x/y columns + exp bias columns."""
    consts = []
    for s, sc in enumerate(SCALES):
        H, W, stride, Q = sc["H"], sc["W"], sc["stride"], sc["Q"]
        HW = H * W
        n = np.arange(HW)
        gx = ((n % W) + 0.5) / W     # (gx + sigmoid(0)-shift)/W with tanh fold
        gy = ((n // W) + 0.5) / H
        c = np.zeros((128, CW), np.float32)
        reps = 128 // Q if s < 2 else 24            # groups stacked on partitions
        c[: reps * Q, 0:J] = np.tile(gx.reshape(Q, J), (reps, 1))
        c[: reps * Q, J:2 * J] = np.tile(gy.reshape(Q, J), (reps, 1))
        for a, (aw, ah) in enumerate(sc["A"]):
            bx = np.log(aw / stride / (2.0 * W))
            by = np.log(ah / stride / (2.0 * H))
            if s < 2:
                c[:, 234 + 2 * a] = bx
                c[:, 235 + 2 * a] = by
            else:                                   # scale 2: row-dependent bias
                r0 = a * BC * Q
                c[r0:r0 + BC * Q, 234] = bx
                c[r0:r0 + BC * Q, 235] = by
        consts.append(c)
    return consts


def _emit_body(tc, in_aps, out_aps, cst_aps, repeat=1):
    nc = tc.nc
    Tn = mybir.ActivationFunctionType.Tanh
    Ex = mybir.ActivationFunctionType.Exp
    # flat element views
    ins = [ap.rearrange("b c h w -> b c (h w)") for ap in in_aps]
    boxes = out_aps[0].rearrange("b n u k -> b n (u k)")
    confs = out_aps[1]
    depth = out_aps[2].rearrange("b n u -> b (n u)")

    with ExitStack() as ctx:
        cpool = ctx.enter_context(tc.tile_pool(name="cst", bufs=1))
        tpool = ctx.enter_context(tc.tile_pool(name="raw", bufs=4))
        vpool = ctx.enter_context(tc.tile_pool(name="act", bufs=4))
        upool = ctx.enter_context(tc.tile_pool(name="uhd", bufs=4))
        spool = ctx.enter_context(tc.tile_pool(name="stg", bufs=4))

        C = []
        for s in range(3):
            c = cpool.tile([128, CW], F32, tag=f"cst{s}")
            nc.sync.dma_start(c[:, :], cst_aps[s][:, :])
            C.append(c)

        for s, groups, Q, P in _tile_plan() * repeat:
            HW = HWS[s]
            W_, H_ = SCALES[s]["W"], SCALES[s]["H"]
            T = tpool.tile([128, 14 * J], F32, tag="T")
            V = tpool.tile([128, 13 * J], F32, tag="V")
            U = upool.tile([128, 2 * J], F32, tag="U")
            HD = upool.tile([128, J], F32, tag="HD")
            SB = spool.tile([128, 4 * J], F32, tag="SB")
            SC = spool.tile([128, CLSN * J], F32, tag="SC")

            # ---- loads: one DMA per (a, b) group, [Q, 14, J] ----
            for g, (a, b) in enumerate(groups):
                src = ins[s][b, 14 * a:14 * (a + 1)].rearrange(
                    "c (q j) -> q c j", j=J)
                dst = T[g * Q:(g + 1) * Q, :].rearrange("q (c j) -> q c j", j=J)
                nc.sync.dma_start(dst, src)

            # ---- activations (one ACT table set: tanh + exp) ----
            nc.scalar.activation(V[:P, 0:2 * J], T[:P, 0:2 * J], Tn, scale=0.5)
            if s < 2:
                a = groups[0][0]   # all groups in tile share the anchor
                bxc, byc = 234 + 2 * a, 235 + 2 * a
            else:
                bxc, byc = 234, 235
            nc.scalar.activation(V[:P, 2 * J:3 * J], T[:P, 2 * J:3 * J], Ex,
                                 bias=C[s][:P, bxc:bxc + 1])
            nc.scalar.activation(V[:P, 3 * J:4 * J], T[:P, 3 * J:4 * J], Ex,
                                 bias=C[s][:P, byc:byc + 1])
            nc.scalar.activation(V[:P, 4 * J:13 * J], T[:P, 4 * J:13 * J], Tn,
                                 scale=0.5)

            # ---- DVE: centers, corners, confidences ----
            nc.vector.scalar_tensor_tensor(
                U[:P, 0:J], V[:P, 0:J], 0.5 / W_, C[s][:P, 0:J],
                op0=AluOpType.mult, op1=AluOpType.add)
            nc.vector.scalar_tensor_tensor(
                U[:P, J:2 * J], V[:P, J:2 * J], 0.5 / H_, C[s][:P, J:2 * J],
                op0=AluOpType.mult, op1=AluOpType.add)

            sb4 = SB[:P, :].rearrange("p (j k) -> p k j", k=4)
            u2 = U[:P, :].rearrange("p (x j) -> p x j", x=2)
            e2 = V[:P, 2 * J:4 * J].rearrange("p (x j) -> p x j", x=2)
            nc.vector.tensor_tensor(sb4[:, 0:2, :], u2, e2,
                                    op=AluOpType.subtract)
            nc.vector.tensor_tensor(sb4[:, 2:4, :], u2, e2, op=AluOpType.add)

            nc.vector.tensor_scalar(HD[:P, :], V[:P, 4 * J:5 * J], 0.25, 0.25,
                                    op0=AluOpType.mult, op1=AluOpType.add)
            sc8 = SC[:P, :].rearrange("p (j c) -> p c j", c=CLSN)
            cl8 = V[:P, 5 * J:13 * J].rearrange("p (c j) -> p c j", c=CLSN)
            hdb = HD[:P, :].unsqueeze(1).broadcast_to((P, CLSN, J))
            nc.vector.scalar_tensor_tensor(
                sc8, cl8, 1.0, hdb, op0=AluOpType.add, op1=AluOpType.mult)

            # ---- stores: merged across same-anchor group runs (2-level
            # partition APs keep each DMA at 3 balanced dims) ----
            for a, b0, ng, g0 in _store_runs(s, groups):
                n0 = NOFF[s] + a * HW
                r = slice(g0 * Q, (g0 + ng) * Q)
                bo = boxes[b0:b0 + ng, n0:n0 + HW, :].rearrange(
                    "g (q j) k -> g q (j k)", j=J)
                nc.sync.dma_start(bo, SB[r, :])
                co = confs[b0:b0 + ng, n0:n0 + HW, :].rearrange(
                    "g (q j) c -> g q (j c)", j=J)
                nc.sync.dma_start(co, SC[r, :])
                de = depth[b0:b0 + ng, n0:n0 + HW].rearrange(
                    "g (q j) -> g q j", j=J)
                nc.sync.dma_start(de, T[r, 13 * J:14 * J])


def _build_nc(repeat=1):
    nc = bacc.Bacc("TRN2", target_bir_lowering=False, debug=False)
    in_aps = [
        nc.dram_tensor(f"out{s + 1}", [BC, 42, SCALES[s]["H"], SCALES[s]["W"]],
                       F32, kind="ExternalInput").ap()
        for s in range(3)
    ]
    cst_aps = [
        nc.dram_tensor(f"cst{s}", [128, CW], F32, kind="ExternalInput").ap()
        for s in range(3)
    ]
    out_aps = [
        nc.dram_tensor("boxes", [BC, NDET, 1, 4], F32,
                       kind="ExternalOutput").ap(),
        nc.dram_tensor("confs", [BC, NDET, CLSN], F32,
                       kind="ExternalOutput").ap(),
        nc.dram_tensor("depth", [BC, NDET, 1], F32,
                       kind="ExternalOutput").ap(),
    ]
    with tile.TileContext(nc) as tc:
        _emit_body(tc, in_aps, out_aps, cst_aps, repeat=repeat)
    nc.compile()
    return nc


_NC_CACHE = None


def _get_nc():
    global _NC_CACHE
    if _NC_CACHE is None:
        _NC_CACHE = _build_nc()
    return _NC_CACHE


def kernel(out1, out2, out3):
    nc = _get_nc()
    consts = _build_consts()
    full = [np.ascontiguousarray(x, dtype=np.float32)
            for x in (out1, out2, out3)]
    in_maps = []
    for i in range(N_CORES):
        m = {f"out{s + 1}": full[s][i * BC:(i + 1) * BC] for s in range(3)}
        for s in range(3):
            m[f"cst{s}"] = consts[s]
        in_maps.append(m)
    res = run_bass_kernel_spmd(nc, in_maps, list(range(N_CORES))).results
    boxes = np.concatenate([res[i]["boxes"] for i in range(N_CORES)], axis=0)
    confs = np.concatenate([res[i]["confs"] for i in range(N_CORES)], axis=0)
    depth = np.concatenate([res[i]["depth"] for i in range(N_CORES)], axis=0)
    return boxes, confs, depth


# revision 17
# speedup vs baseline: 1.4297x; 1.4297x over previous
"""Trainium2 Bass kernel for the YOLO-style DetectHead decode (nms_detection).

Contract: kernel(out1, out2, out3) -> (boxes, confs, depth), matching the
jax reference decode exactly (full 64-batch inputs in, full outputs out).

Strategy
--------
Pure data parallel over 8 NeuronCores: 8 batches per core. On-core, one
SBUF tile per (batch, scale): partition rows are (anchor, q), each row
holding J consecutive detections (HW = Q * J exactly, zero padding).
Because rows sweep (a, q) in detection order, every output store is ONE
fully-contiguous 2-D DMA ([P, J*k] SBUF -> flat DRAM), and loads are one
3-D DMA per anchor with >=512B contiguous runs for the large scales.
The [N,4]/[N,8] interleaved outputs are written directly by strided DVE
writes - no transposes anywhere.

Sigmoid is computed as 0.5*tanh(x/2)+0.5 (tanh and exp share one ACT
table set; sigmoid+exp would force a ~2.7us table switch per op), with
all affine pieces folded into fused DVE ops using per-partition scalar
columns from a constant block:
  u      = tanh(tx/2)*(0.5/W) + (gx+0.5)/W          (grid constant)
  x1,x2  = exp(tw) * (-/+ ax/(2W)) + u              (anchor as scalar AP)
  conf   = (tanh(cls/2) + 1) * (0.25*tanh(det/2)+0.25)
  depth  = raw copy (pure DMA from the loaded tile)
"""

from contextlib import ExitStack

import numpy as np

import concourse.bass as bass
import concourse.tile as tile
from concourse import bacc, mybir
from concourse.alu_op_type import AluOpType
from concourse.bass_utils import run_bass_kernel_spmd

N_CORES = 8
BC = 8          # batches per core (64 / 8)
CLSN = 8        # num classes
F32 = mybir.dt.float32

# per scale: H, W, stride, anchors, Qp (q rows per anchor), J (dets/row)
SCALES = [
    dict(H=48, W=156, stride=8,  A=[(12, 16), (19, 36), (40, 28)],
         Qp=39, J=192),
    dict(H=24, W=78,  stride=16, A=[(36, 75), (76, 55), (72, 146)],
         Qp=13, J=144),
    dict(H=12, W=39,  stride=32, A=[(142, 110), (192, 243), (459, 401)],
         Qp=39, J=12),
]
HWS = [s["H"] * s["W"] for s in SCALES]          # 7488, 1872, 468
NDET = 3 * sum(HWS)                              # 29484
NOFF = [0, 3 * HWS[0], 3 * HWS[0] + 3 * HWS[1]]  # per-scale offset in N
# const block column offsets: per scale [gx J | gy J | kxn kxp kyn kyp]
COFF = []
_o = 0
for _s in SCALES:
    COFF.append(_o)
    _o += 2 * _s["J"] + 4
CW = _o                                          # 388 + 292 + 28 = 708


def _build_consts():
    """[128, CW] f32 const block per the COFF layout (shared by all cores)."""
    c = np.zeros((128, CW), np.float32)
    for s, sc in enumerate(SCALES):
        H, W, stride, Qp, J = sc["H"], sc["W"], sc["stride"], sc["Qp"], sc["J"]
        HW = H * W
        P = 3 * Qp
        o = COFF[s]
        n = np.arange(HW)
        gx = (((n % W) + 0.5) / W).reshape(Qp, J)
        gy = (((n // W) + 0.5) / H).reshape(Qp, J)
        c[:P, o:o + J] = np.tile(gx, (3, 1))
        c[:P, o + J:o + 2 * J] = np.tile(gy, (3, 1))
        for a, (aw, ah) in enumerate(sc["A"]):
            kx = aw / stride / (2.0 * W)
            ky = ah / stride / (2.0 * H)
            r = slice(a * Qp, (a + 1) * Qp)
            c[r, o + 2 * J + 0] = -kx
            c[r, o + 2 * J + 1] = kx
            c[r, o + 2 * J + 2] = -ky
            c[r, o + 2 * J + 3] = ky
    return c


def _emit_body(tc, in_aps, out_aps, cst_ap, repeat=1):
    nc = tc.nc
    Tn = mybir.ActivationFunctionType.Tanh
    Ex = mybir.ActivationFunctionType.Exp
    ins = [ap.rearrange("b c h w -> b c (h w)") for ap in in_aps]
    boxes = out_aps[0].rearrange("b n u k -> b (n u k)")
    confs = out_aps[1].rearrange("b n c -> b (n c)")
    depth = out_aps[2].rearrange("b n u -> b (n u)")

    with ExitStack() as ctx:
        cpool = ctx.enter_context(tc.tile_pool(name="cst", bufs=1))
        tpool = ctx.enter_context(tc.tile_pool(name="raw", bufs=3))
        vpool = ctx.enter_context(tc.tile_pool(name="act", bufs=3))
        upool = ctx.enter_context(tc.tile_pool(name="uhd", bufs=3))
        spool = ctx.enter_context(tc.tile_pool(name="stg", bufs=3))

        CST = cpool.tile([128, CW], F32, tag="cst")
        nc.sync.dma_start(CST[:, :], cst_ap[:, :])

        plan = [(b, s) for s in range(3) for b in range(BC)]
        for b, s in plan * repeat:
            sc = SCALES[s]
            HW, W_, H_ = HWS[s], sc["W"], sc["H"]
            Qp, J = sc["Qp"], sc["J"]
            P = 3 * Qp
            o = COFF[s]
            T = tpool.tile([128, 14 * J], F32, tag=f"T{s}")
            V = vpool.tile([128, 13 * J], F32, tag=f"V{s}")
            U = upool.tile([128, J], F32, tag=f"U{s}")
            HD = upool.tile([128, J], F32, tag=f"HD{s}")
            SB = spool.tile([128, 4 * J], F32, tag=f"SB{s}")
            SC = spool.tile([128, CLSN * J], F32, tag=f"SC{s}")

            # ---- loads: one 3-D DMA per anchor ----
            for a in range(3):
                src = ins[s][b, 14 * a:14 * (a + 1)].rearrange(
                    "c (q j) -> q c j", j=J)
                dst = T[a * Qp:(a + 1) * Qp, :].rearrange(
                    "q (c j) -> q c j", j=J)
                nc.sync.dma_start(dst, src)

            # ---- activations: 3 ops, one table set ----
            nc.scalar.activation(V[:P, 0:2 * J], T[:P, 0:2 * J], Tn, scale=0.5)
            nc.scalar.activation(V[:P, 2 * J:4 * J], T[:P, 2 * J:4 * J], Ex)
            nc.scalar.activation(V[:P, 4 * J:13 * J], T[:P, 4 * J:13 * J], Tn,
                                 scale=0.5)

            # ---- DVE ----
            # u_x lives in U, u_y in HD's former slot? keep two tiles:
            nc.vector.scalar_tensor_tensor(
                U[:P, :], V[:P, 0:J], 0.5 / W_, CST[:P, o:o + J],
                op0=AluOpType.mult, op1=AluOpType.add)
            UY = HD  # reuse later after corners; separate tag below
            nc.vector.scalar_tensor_tensor(
                UY[:P, :], V[:P, J:2 * J], 0.5 / H_, CST[:P, o + J:o + 2 * J],
                op0=AluOpType.mult, op1=AluOpType.add)

            sb4 = SB[:P, :].rearrange("p (j k) -> p k j", k=4)
            ew = V[:P, 2 * J:3 * J]
            eh = V[:P, 3 * J:4 * J]
            kc = o + 2 * J
            # corner = exp * (+-k) + u   (k is a per-partition scalar AP)
            nc.vector.scalar_tensor_tensor(
                sb4[:, 0, :], ew, CST[:P, kc + 0:kc + 1], U[:P, :],
                op0=AluOpType.mult, op1=AluOpType.add)
            nc.vector.scalar_tensor_tensor(
                sb4[:, 1, :], eh, CST[:P, kc + 2:kc + 3], UY[:P, :],
                op0=AluOpType.mult, op1=AluOpType.add)
            nc.vector.scalar_tensor_tensor(
                sb4[:, 2, :], ew, CST[:P, kc + 1:kc + 2], U[:P, :],
                op0=AluOpType.mult, op1=AluOpType.add)
            nc.vector.scalar_tensor_tensor(
                sb4[:, 3, :], eh, CST[:P, kc + 3:kc + 4], UY[:P, :],
                op0=AluOpType.mult, op1=AluOpType.add)

            # conf = (tanh_cls + 1) * (0.25*tanh_det + 0.25)
            HD2 = upool.tile([128, J], F32, tag=f"HD2{s}")
            nc.vector.tensor_scalar(HD2[:P, :], V[:P, 4 * J:5 * J], 0.25, 0.25,
                                    op0=AluOpType.mult, op1=AluOpType.add)
            sc8 = SC[:P, :].rearrange("p (j c) -> p c j", c=CLSN)
            cl8 = V[:P, 5 * J:13 * J].rearrange("p (c j) -> p c j", c=CLSN)
            hdb = HD2[:P, :].unsqueeze(1).broadcast_to((P, CLSN, J))
            nc.vector.scalar_tensor_tensor(
                sc8, cl8, 1.0, hdb, op0=AluOpType.add, op1=AluOpType.mult)

            # ---- stores: one contiguous 2-D DMA per output ----
            e0 = NOFF[s]
            bo = boxes[b, 4 * e0:4 * (e0 + 3 * HW)].rearrange(
                "(p f) -> p f", p=P)
            nc.sync.dma_start(bo, SB[:P, :])
            co = confs[b, 8 * e0:8 * (e0 + 3 * HW)].rearrange(
                "(p f) -> p f", p=P)
            nc.sync.dma_start(co, SC[:P, :])
            de = depth[b, e0:e0 + 3 * HW].rearrange("(p f) -> p f", p=P)
            nc.sync.dma_start(de, T[:P, 13 * J:14 * J])


def _build_nc(repeat=1):
    nc = bacc.Bacc("TRN2", target_bir_lowering=False, debug=False)
    in_aps = [
        nc.dram_tensor(f"out{s + 1}", [BC, 42, SCALES[s]["H"], SCALES[s]["W"]],
                       F32, kind="ExternalInput").ap()
        for s in range(3)
    ]
    cst_ap = nc.dram_tensor("cst", [128, CW], F32, kind="ExternalInput").ap()
    out_aps = [
        nc.dram_tensor("boxes", [BC, NDET, 1, 4], F32,
                       kind="ExternalOutput").ap(),
        nc.dram_tensor("confs", [BC, NDET, CLSN], F32,
                       kind="ExternalOutput").ap(),
        nc.dram_tensor("depth", [BC, NDET, 1], F32,
                       kind="ExternalOutput").ap(),
    ]
    with tile.TileContext(nc) as tc:
        _emit_body(tc, in_aps, out_aps, cst_ap, repeat=repeat)
    nc.compile()
    return nc


_NC_CACHE = None


def _get_nc():
    global _NC_CACHE
    if _NC_CACHE is None:
        _NC_CACHE = _build_nc()
    return _NC_CACHE


def kernel(out1, out2, out3):
    nc = _get_nc()
    cst = _build_consts()
    full = [np.ascontiguousarray(x, dtype=np.float32)
            for x in (out1, out2, out3)]
    in_maps = []
    for i in range(N_CORES):
        m = {f"out{s + 1}": full[s][i * BC:(i + 1) * BC] for s in range(3)}
        m["cst"] = cst
        in_maps.append(m)
    res = run_bass_kernel_spmd(nc, in_maps, list(range(N_CORES))).results
    boxes = np.concatenate([res[i]["boxes"] for i in range(N_CORES)], axis=0)
    confs = np.concatenate([res[i]["confs"] for i in range(N_CORES)], axis=0)
    depth = np.concatenate([res[i]["depth"] for i in range(N_CORES)], axis=0)
    return boxes, confs, depth


# revision 19
# speedup vs baseline: 4.4766x; 3.1312x over previous
"""Trainium2 Bass kernel for the YOLO-style DetectHead decode (nms_detection).

Contract: kernel(out1, out2, out3) -> (boxes, confs, depth), matching the
jax reference decode exactly (full 64-batch inputs in, full outputs out).

Strategy
--------
Pure data parallel over 8 NeuronCores: 8 batches per core. On-core, one
SBUF tile per (batch, scale): partition rows are (anchor, q), each row
holding J consecutive detections (HW = Q * J exactly, zero padding).
Because rows sweep (a, q) in detection order, every output store is ONE
fully-contiguous 2-D DMA ([P, J*k] SBUF -> flat DRAM), and loads are one
3-D DMA per anchor with >=512B contiguous runs for the large scales.
The [N,4]/[N,8] interleaved outputs are written directly by strided DVE
writes - no transposes anywhere.

Sigmoid is computed as 0.5*tanh(x/2)+0.5 (tanh and exp share one ACT
table set; sigmoid+exp would force a ~2.7us table switch per op), with
all affine pieces folded into fused DVE ops using per-partition scalar
columns from a constant block:
  u      = tanh(tx/2)*(0.5/W) + (gx+0.5)/W          (grid constant)
  x1,x2  = exp(tw) * (-/+ ax/(2W)) + u              (anchor as scalar AP)
  conf   = (tanh(cls/2) + 1) * (0.25*tanh(det/2)+0.25)
  depth  = raw copy (pure DMA from the loaded tile)
"""

from contextlib import ExitStack

import numpy as np

import concourse.bass as bass
import concourse.tile as tile
from concourse import bacc, mybir
from concourse.alu_op_type import AluOpType
from concourse.bass_utils import run_bass_kernel_spmd

N_CORES = 8
BC = 8          # batches per core (64 / 8)
CLSN = 8        # num classes
F32 = mybir.dt.float32

# per scale: H, W, stride, anchors, Qp (q rows per anchor), J (dets/row)
SCALES = [
    dict(H=48, W=156, stride=8,  A=[(12, 16), (19, 36), (40, 28)],
         Qp=39, J=192),
    dict(H=24, W=78,  stride=16, A=[(36, 75), (76, 55), (72, 146)],
         Qp=13, J=144),
    dict(H=12, W=39,  stride=32, A=[(142, 110), (192, 243), (459, 401)],
         Qp=39, J=12),
]
HWS = [s["H"] * s["W"] for s in SCALES]          # 7488, 1872, 468
NDET = 3 * sum(HWS)                              # 29484
NOFF = [0, 3 * HWS[0], 3 * HWS[0] + 3 * HWS[1]]  # per-scale offset in N
# const block column offsets: per scale [gx J | gy J | kxn kxp kyn kyp]
COFF = []
_o = 0
for _s in SCALES:
    COFF.append(_o)
    _o += 2 * _s["J"] + 4
CW = _o                                          # 388 + 292 + 28 = 708


def _build_consts():
    """[128, CW] f32 const block per the COFF layout (shared by all cores)."""
    c = np.zeros((128, CW), np.float32)
    for s, sc in enumerate(SCALES):
        H, W, stride, Qp, J = sc["H"], sc["W"], sc["stride"], sc["Qp"], sc["J"]
        HW = H * W
        P = 3 * Qp
        o = COFF[s]
        n = np.arange(HW)
        gx = (((n % W) + 0.5) / W).reshape(Qp, J)
        gy = (((n // W) + 0.5) / H).reshape(Qp, J)
        c[:P, o:o + J] = np.tile(gx, (3, 1))
        c[:P, o + J:o + 2 * J] = np.tile(gy, (3, 1))
        for a, (aw, ah) in enumerate(sc["A"]):
            kx = aw / stride / (2.0 * W)
            ky = ah / stride / (2.0 * H)
            r = slice(a * Qp, (a + 1) * Qp)
            c[r, o + 2 * J + 0] = -kx
            c[r, o + 2 * J + 1] = kx
            c[r, o + 2 * J + 2] = -ky
            c[r, o + 2 * J + 3] = ky
    return c


def _emit_body(tc, in_aps, out_aps, cst_ap, repeat=1, mode="full"):
    nc = tc.nc
    Tn = mybir.ActivationFunctionType.Tanh
    Ex = mybir.ActivationFunctionType.Exp
    ins = [ap.rearrange("b c h w -> b c (h w)") for ap in in_aps]
    boxes = out_aps[0].rearrange("b n u k -> b (n u k)")
    confs = out_aps[1].rearrange("b n c -> b (n c)")
    depth = out_aps[2].rearrange("b n u -> b (n u)")

    with ExitStack() as ctx:
        cpool = ctx.enter_context(tc.tile_pool(name="cst", bufs=1))
        tpool = ctx.enter_context(tc.tile_pool(name="raw", bufs=3))
        vpool = ctx.enter_context(tc.tile_pool(name="act", bufs=3))
        upool = ctx.enter_context(tc.tile_pool(name="uhd", bufs=3))
        spool = ctx.enter_context(tc.tile_pool(name="stg", bufs=3))

        CST = cpool.tile([128, CW], F32, tag="cst")
        nc.sync.dma_start(CST[:, :], cst_ap[:, :])

        plan = [(b, s) for s in range(3) for b in range(BC)]
        for b, s in plan * repeat:
            sc = SCALES[s]
            HW, W_, H_ = HWS[s], sc["W"], sc["H"]
            Qp, J = sc["Qp"], sc["J"]
            P = 3 * Qp
            o = COFF[s]
            T = tpool.tile([128, 14 * J], F32, tag=f"T{s}")
            V = vpool.tile([128, 13 * J], F32, tag=f"V{s}")
            U = upool.tile([128, J], F32, tag=f"U{s}")
            HD = upool.tile([128, J], F32, tag=f"HD{s}")
            SB = spool.tile([128, 4 * J], F32, tag=f"SB{s}")
            SC = spool.tile([128, CLSN * J], F32, tag=f"SC{s}")

            # ---- loads: one 3-D DMA per anchor ----
            for a in range(3):
                src = ins[s][b, 14 * a:14 * (a + 1)].rearrange(
                    "c (q j) -> q c j", j=J)
                dst = T[a * Qp:(a + 1) * Qp, :].rearrange(
                    "q (c j) -> q c j", j=J)
                nc.sync.dma_start(dst, src)

            # ---- activations: 3 ops, one table set ----
            if mode == "dma":
                # stores read raw T data (same sizes); skip all compute
                e0 = NOFF[s]
                bo = boxes[b, 4 * e0:4 * (e0 + 3 * HW)].rearrange(
                    "(p f) -> p f", p=P)
                nc.sync.dma_start(bo, T[:P, 0:4 * J])
                co = confs[b, 8 * e0:8 * (e0 + 3 * HW)].rearrange(
                    "(p f) -> p f", p=P)
                nc.sync.dma_start(co, T[:P, 4 * J:12 * J])
                de = depth[b, e0:e0 + 3 * HW].rearrange("(p f) -> p f", p=P)
                nc.sync.dma_start(de, T[:P, 13 * J:14 * J])
                continue
            nc.scalar.activation(V[:P, 0:2 * J], T[:P, 0:2 * J], Tn, scale=0.5)
            nc.scalar.activation(V[:P, 2 * J:4 * J], T[:P, 2 * J:4 * J], Ex)
            nc.scalar.activation(V[:P, 4 * J:13 * J], T[:P, 4 * J:13 * J], Tn,
                                 scale=0.5)

            # ---- DVE ----
            # u_x lives in U, u_y in HD's former slot? keep two tiles:
            nc.vector.scalar_tensor_tensor(
                U[:P, :], V[:P, 0:J], 0.5 / W_, CST[:P, o:o + J],
                op0=AluOpType.mult, op1=AluOpType.add)
            UY = HD  # reuse later after corners; separate tag below
            nc.vector.scalar_tensor_tensor(
                UY[:P, :], V[:P, J:2 * J], 0.5 / H_, CST[:P, o + J:o + 2 * J],
                op0=AluOpType.mult, op1=AluOpType.add)

            sb4 = SB[:P, :].rearrange("p (j k) -> p k j", k=4)
            ew = V[:P, 2 * J:3 * J]
            eh = V[:P, 3 * J:4 * J]
            kc = o + 2 * J
            # corner = exp * (+-k) + u   (k is a per-partition scalar AP)
            nc.vector.scalar_tensor_tensor(
                sb4[:, 0, :], ew, CST[:P, kc + 0:kc + 1], U[:P, :],
                op0=AluOpType.mult, op1=AluOpType.add)
            nc.vector.scalar_tensor_tensor(
                sb4[:, 1, :], eh, CST[:P, kc + 2:kc + 3], UY[:P, :],
                op0=AluOpType.mult, op1=AluOpType.add)
            nc.vector.scalar_tensor_tensor(
                sb4[:, 2, :], ew, CST[:P, kc + 1:kc + 2], U[:P, :],
                op0=AluOpType.mult, op1=AluOpType.add)
            nc.vector.scalar_tensor_tensor(
                sb4[:, 3, :], eh, CST[:P, kc + 3:kc + 4], UY[:P, :],
                op0=AluOpType.mult, op1=AluOpType.add)

            # conf = (tanh_cls + 1) * (0.25*tanh_det + 0.25)
            HD2 = upool.tile([128, J], F32, tag=f"HD2{s}")
            nc.vector.tensor_scalar(HD2[:P, :], V[:P, 4 * J:5 * J], 0.25, 0.25,
                                    op0=AluOpType.mult, op1=AluOpType.add)
            sc8 = SC[:P, :].rearrange("p (j c) -> p c j", c=CLSN)
            cl8 = V[:P, 5 * J:13 * J].rearrange("p (c j) -> p c j", c=CLSN)
            hdb = HD2[:P, :].unsqueeze(1).broadcast_to((P, CLSN, J))
            nc.vector.scalar_tensor_tensor(
                sc8, cl8, 1.0, hdb, op0=AluOpType.add, op1=AluOpType.mult)

            # ---- stores: one contiguous 2-D DMA per output ----
            if mode == "nostore":
                continue
            e0 = NOFF[s]
            bo = boxes[b, 4 * e0:4 * (e0 + 3 * HW)].rearrange(
                "(p f) -> p f", p=P)
            nc.sync.dma_start(bo, SB[:P, :])
            co = confs[b, 8 * e0:8 * (e0 + 3 * HW)].rearrange(
                "(p f) -> p f", p=P)
            nc.sync.dma_start(co, SC[:P, :])
            de = depth[b, e0:e0 + 3 * HW].rearrange("(p f) -> p f", p=P)
            nc.sync.dma_start(de, T[:P, 13 * J:14 * J])


def _build_nc(repeat=1, mode="full"):
    nc = bacc.Bacc("TRN2", target_bir_lowering=False, debug=False)
    in_aps = [
        nc.dram_tensor(f"out{s + 1}", [BC, 42, SCALES[s]["H"], SCALES[s]["W"]],
                       F32, kind="ExternalInput").ap()
        for s in range(3)
    ]
    cst_ap = nc.dram_tensor("cst", [128, CW], F32, kind="ExternalInput").ap()
    out_aps = [
        nc.dram_tensor("boxes", [BC, NDET, 1, 4], F32,
                       kind="ExternalOutput").ap(),
        nc.dram_tensor("confs", [BC, NDET, CLSN], F32,
                       kind="ExternalOutput").ap(),
        nc.dram_tensor("depth", [BC, NDET, 1], F32,
                       kind="ExternalOutput").ap(),
    ]
    with tile.TileContext(nc) as tc:
        _emit_body(tc, in_aps, out_aps, cst_ap, repeat=repeat, mode=mode)
    nc.compile()
    return nc


_NC_CACHE = None


def _get_nc():
    global _NC_CACHE
    if _NC_CACHE is None:
        _NC_CACHE = _build_nc()
    return _NC_CACHE


def kernel(out1, out2, out3):
    nc = _get_nc()
    cst = _build_consts()
    full = [np.ascontiguousarray(x, dtype=np.float32)
            for x in (out1, out2, out3)]
    in_maps = []
    for i in range(N_CORES):
        m = {f"out{s + 1}": full[s][i * BC:(i + 1) * BC] for s in range(3)}
        m["cst"] = cst
        in_maps.append(m)
    res = run_bass_kernel_spmd(nc, in_maps, list(range(N_CORES))).results
    boxes = np.concatenate([res[i]["boxes"] for i in range(N_CORES)], axis=0)
    confs = np.concatenate([res[i]["confs"] for i in range(N_CORES)], axis=0)
    depth = np.concatenate([res[i]["depth"] for i in range(N_CORES)], axis=0)
    return boxes, confs, depth
